# revision 15
# baseline (speedup 1.0000x reference)
"""BitNet DiT on 8 Trainium2 NeuronCores — data-parallel over batch (2 images/core).

Host: patchify, time-embedding + adaLN modulation vectors, BitNet weight
quantization (ternary * per-tensor scale) -> bf16 upload.
Device: full 12-block DiT forward per core in a single Bass/Tile kernel.
BitNet matmuls run as exact integer arithmetic in bf16 (|values| <= 127,
fp32 accumulate). Attention runs in fp32 via transposed-logits + ones-column
softmax-denominator trick.

v2: activation transposes ride the DMA xbar (dma_start_transpose) instead of
PE+DVE; fused scalar_tensor_tensor for modulation and evac+residual;
in-place magic rounding; rstd via Sqrt+reciprocal (no ACT table thrash);
software-pipelined attention heads.
"""
import math
import os
import sys
import numpy as np

sys.path.insert(0, "/opt/trn_rl_repo")

import ml_dtypes  # noqa: E402
import concourse.bass as bass  # noqa: E402
import concourse.mybir as mybir  # noqa: E402
import concourse.tile as tile  # noqa: E402
from concourse import bacc  # noqa: E402
from concourse.bass_utils import run_bass_kernel_spmd  # noqa: E402
from concourse.masks import make_identity  # noqa: E402

F32 = mybir.dt.float32
F32R = mybir.dt.float32r
FP8 = mybir.dt.float8e4
BF16 = mybir.dt.bfloat16
AX = mybir.AxisListType
OP = mybir.AluOpType
AF = mybir.ActivationFunctionType

DIM = 768
DEPTH = int(os.environ.get("KERNEL_DEPTH", "12"))
HEADS = 12
HD = 64
PATCH = 16
IMG = 256
CIN = 3
HID = 4 * DIM
EPS = 1e-6
P = 128
T = 512            # tokens per core (2 images x 256)
NT = T // P        # 4 token tiles
NTOK = 256         # tokens per image
KD = DIM // P      # 6
KH = HID // P      # 24
MAGIC = float(np.float32(3 * 2**22))  # 12582912.0 RNE round-to-int magic

_CACHED = {}


def _mm_chunks(n):
    out = []
    s = 0
    while s < n:
        e = min(s + 512, n)
        out.append((s, e))
        s = e
    return out


def build_program(depth=DEPTH):
    nc = bacc.Bacc("TRN2", target_bir_lowering=False, debug=False, num_devices=8)

    xpT_d = nc.declare_dram_parameter("xpT", [DIM, T], F32, isOutput=False)
    posb_d = nc.declare_dram_parameter("posb", [NTOK, DIM], F32, isOutput=False)
    patchWT_d = nc.declare_dram_parameter("patchWT", [DIM, DIM], F32, isOutput=False)
    headWT_d = nc.declare_dram_parameter("headWT", [DIM, DIM], F32, isOutput=False)
    headb_d = nc.declare_dram_parameter("headb", [1, DIM], F32, isOutput=False)
    wqkv_d = nc.declare_dram_parameter("wqkv", [depth, DIM, 3 * DIM], FP8, isOutput=False)
    wproj_d = nc.declare_dram_parameter("wproj", [depth, DIM, DIM], FP8, isOutput=False)
    wfc1_d = nc.declare_dram_parameter("wfc1", [depth, DIM, HID], FP8, isOutput=False)
    wfc2_d = nc.declare_dram_parameter("wfc2", [depth, HID, DIM], FP8, isOutput=False)
    # modulation vectors: [block, norm(2), part, img(2), A/B(2), 768]
    mods_d = nc.declare_dram_parameter("mods", [depth, 2, P, 2, 2, DIM], F32, isOutput=False)
    wscl_d = nc.declare_dram_parameter("wscl", [1, 4 * depth], F32, isOutput=False)
    out_d = nc.declare_dram_parameter("zout", [T, DIM], F32, isOutput=True)

    with tile.TileContext(nc) as tc:
        from contextlib import ExitStack
        with ExitStack() as _ctx:
            constp = _ctx.enter_context(tc.tile_pool(name="const", bufs=1))
            residp = _ctx.enter_context(tc.tile_pool(name="resid", bufs=1))
            fm6p = _ctx.enter_context(tc.tile_pool(name="fm6", bufs=2))
            wp = _ctx.enter_context(tc.tile_pool(name="w", bufs=6))
            modp = _ctx.enter_context(tc.tile_pool(name="mod", bufs=2))
            tmp_ = _ctx.enter_context(tc.tile_pool(name="tm", bufs=1))
            gp = _ctx.enter_context(tc.tile_pool(name="g", bufs=4))
            hp = _ctx.enter_context(tc.tile_pool(name="h", bufs=5))
            xqp6 = _ctx.enter_context(tc.tile_pool(name="xqp6", bufs=3))
            xqp24 = _ctx.enter_context(tc.tile_pool(name="xqp24", bufs=2))
            xq6p = _ctx.enter_context(tc.tile_pool(name="xq6", bufs=7))
            xq24p = _ctx.enter_context(tc.tile_pool(name="xq24", bufs=2))
            eTp = _ctx.enter_context(tc.tile_pool(name="eT", bufs=2))
            scp = _ctx.enter_context(tc.tile_pool(name="sc", bufs=64))
            ps_mm = _ctx.enter_context(tc.tile_pool(name="ps_mm", bufs=2, space="PSUM"))
            ps_tp = _ctx.enter_context(tc.tile_pool(name="ps_tp", bufs=2, space="PSUM"))
            ps_lt = _ctx.enter_context(tc.tile_pool(name="ps_lt", bufs=2, space="PSUM"))
            ps_oa = _ctx.enter_context(tc.tile_pool(name="ps_oa", bufs=2, space="PSUM"))

            idf = constp.tile([P, P], F32)
            make_identity(nc, idf[:])

            # broadcast w_scales/127 to all partitions
            wsrow = constp.tile([1, 4 * depth], F32)
            nc.sync.dma_start(wsrow[:], wscl_d[:])
            wsb = constp.tile([P, 4 * depth], F32)
            nc.gpsimd.partition_broadcast(wsb[:], wsrow[0:1, :])
            nmag = constp.tile([P, 1], F32)
            nc.vector.memset(nmag[:], -MAGIC)
            pmag = constp.tile([P, 1], F32)
            nc.vector.memset(pmag[:], MAGIC)

            z = residp.tile([P, NT, DIM], F32)
            v_aug = residp.tile([P, NT, HEADS, HD + 1], F32)
            nc.vector.memset(v_aug[:, :, :, HD], 1.0)
            o_tm = residp.tile([P, NT, DIM], F32)

            # ---------------- patch embed ----------------
            posb_sb = gp.tile([P, 2, DIM], F32, tag="g", name="posb_sb")
            nc.sync.dma_start(posb_sb[:], posb_d.rearrange("(a p) d -> p a d", p=P))
            xpT = fm6p.tile([P, KD, T], F32, tag="fm6")
            nc.sync.dma_start(xpT[:], xpT_d.rearrange("(o p) t -> p o t", p=P))
            pw_pieces = []
            for i in range(3):
                pwp = gp.tile([P, 2, DIM], F32, tag="g", name="pwp")
                nc.gpsimd.dma_start(
                    pwp[:], patchWT_d[i * 2 * P:(i + 1) * 2 * P, :].rearrange(
                        "(o p) d -> p o d", p=P))
                pw_pieces.append(pwp)
            for t in range(NT):
                for (cs, ce) in _mm_chunks(DIM):
                    pt = ps_mm.tile([P, 512], F32, tag="mm", name="pmm")[:, : ce - cs]
                    for k in range(KD):
                        nc.tensor.matmul(pt[:], xpT[:, k, t * P:(t + 1) * P],
                                         pw_pieces[k // 2][:, k % 2, cs:ce],
                                         start=(k == 0), stop=(k == KD - 1))
                    nc.vector.tensor_tensor(z[:, t, cs:ce], pt[:], posb_sb[:, t % 2, cs:ce], OP.add)

            def load_w(dram, b, kchunks, width, npieces):
                """Stage one linear's transposed fp8 weights as npieces tiles."""
                span = kchunks // npieces
                tiles = []
                for i in range(npieces):
                    wt = wp.tile([P, span, width], FP8, tag="w")
                    nc.gpsimd.dma_start(
                        wt[:],
                        dram[b, i * span * P:(i + 1) * span * P, :].rearrange(
                            "(o p) f -> p o f", p=P))
                    tiles.append(wt)
                return tiles, span

            def quant_smalls(src_ap, ws_idx):
                """Per-token quant scales from one [128,w] source.
                Returns (s127, c)."""
                amax = scp.tile([P, 1], F32, tag="sc", name="amax")
                nc.vector.tensor_reduce(amax[:], src_ap, axis=AX.X, op=OP.max,
                                        apply_absolute_value=True)
                ac = scp.tile([P, 1], F32, tag="sc", name="amaxc")
                nc.vector.tensor_scalar_max(ac[:], amax[:], 1e-5)
                rs = scp.tile([P, 1], F32, tag="sc", name="rcp")
                nc.vector.reciprocal(rs[:], ac[:])
                s127 = scp.tile([P, 1], F32, tag="sc", name="s127")
                nc.vector.tensor_scalar_mul(s127[:], rs[:], 127.0)
                c = scp.tile([P, 1], F32, tag="sc", name="cc")
                nc.vector.tensor_scalar(c[:], ac[:], wsb[:, ws_idx:ws_idx + 1],
                                        None, OP.mult)
                return s127, c

            def quant_round_dma(src_ap, kchunks, s127):
                """In-place magic-round src*(s127) on ACT, unmagic to bf16 (ACT),
                DMA-transpose. Returns the transposed xqT tile [P, KD, 128]."""
                nc.scalar.activation(src_ap, src_ap, AF.Identity, scale=s127[:],
                                     bias=pmag[:])
                xq = xqp6.tile([P, DIM], BF16, tag="xqp6", name="xq6s")
                dst = xq6p.tile([P, KD, P], BF16, tag="xq6")
                nc.scalar.activation(xq[:], src_ap, AF.Identity, bias=nmag[:])
                nc.sync.dma_start_transpose(dst[:], xq[:])
                return dst

            def quant_round_dma_hid(ghalves, s127):
                """HID version: two [P,1536] halves, unmagic on DVE."""
                dst = xq24p.tile([P, KH, P], BF16, tag="xq24")
                for i, gh in enumerate(ghalves):
                    nc.scalar.activation(gh[:], gh[:], AF.Identity, scale=s127[:],
                                         bias=pmag[:])
                    xq = xqp24.tile([P, HID // 2], BF16, tag="xqp24", name="xq24s")
                    nc.vector.tensor_scalar(xq[:], gh[:], MAGIC, None, OP.subtract)
                    nc.sync.dma_start_transpose(dst[:, i * 12:(i + 1) * 12, :], xq[:])
                return dst

            def rstd_from_ssq(ssq):
                ms = scp.tile([P, 1], F32, tag="sc", name="msn")
                nc.vector.tensor_scalar(ms[:], ssq[:], 1.0 / DIM, EPS, OP.mult, OP.add)
                srt = scp.tile([P, 1], F32, tag="sc", name="srt")
                nc.scalar.activation(srt[:], ms[:], AF.Sqrt)
                rst = scp.tile([P, 1], F32, tag="sc", name="rstn")
                nc.vector.reciprocal(rst[:], srt[:])
                return rst

            def norm_mod(t, mt, rstd, dst):
                """dst = (z[t]*rstd) * modA + modB  (2 DVE ops via STT)."""
                img = t // 2
                nc.vector.scalar_tensor_tensor(dst, z[:, t, :], rstd[:],
                                               mt[:, img, 0, :], OP.mult, OP.mult)
                nc.gpsimd.tensor_tensor(dst, dst, mt[:, img, 1, :], OP.add)

            # ---- prologue: phase 1 of block 0 ----
            def load_mods(b_, n_, name):
                mt = modp.tile([P, 2, 2, DIM], F32, tag="mod", name=name)
                nc.gpsimd.dma_start(mt[:], mods_d[b_, n_])
                return mt

            mt1_nxt = load_mods(0, 0, "mt1")
            xq1_cur = [None] * NT
            cq8_cur = [None] * NT
            c_cur = [None] * NT
            ssq0 = [None] * NT
            sq_scr = tmp_.tile([P, DIM], F32, tag="tm", name="sqscr")
            for t in range(NT):
                sv = scp.tile([P, 1], F32, tag="sc", name="ssq")
                nc.scalar.activation(sq_scr[:], z[:, t, :], AF.Square, accum_out=sv[:])
                ssq0[t] = sv
            for t in range(NT):
                rst = rstd_from_ssq(ssq0[t])
                h = hp.tile([P, DIM], F32, tag="h")
                norm_mod(t, mt1_nxt, rst, h[:])
                s127, c = quant_smalls(h[:], 0)
                c_cur[t] = c
                cq8 = scp.tile([P, 1], F32, tag="sc", name="cq8")
                nc.vector.tensor_scalar_mul(cq8[:], c[:], 0.125)
                cq8_cur[t] = cq8
                xq1_cur[t] = quant_round_dma(h[:], KD, s127)

            for b in range(depth):
                xq1s, cq8s, c_list = xq1_cur, cq8_cur, c_cur
                mt2 = load_mods(b, 1, "mt2")

                wq_tiles, wq_half = load_w(wqkv_d, b, KD, 3 * DIM, 2)
                q_fm = fm6p.tile([P, KD, T], F32R, tag="fm6")
                k_fm = fm6p.tile([P, KD, T], F32R, tag="fm6")

                # --- phase 2: qkv + q/k transposes (pipelined by one tile) ---
                q_tms = [None] * NT
                k_tms = [None] * NT

                def p2_mm(t):
                    q_tm = hp.tile([P, DIM], F32, tag="h", name="q_tm")
                    k_tm = hp.tile([P, DIM], F32, tag="h", name="k_tm")
                    q_tms[t], k_tms[t] = q_tm, k_tm
                    for (cs, ce) in _mm_chunks(3 * DIM):
                        pt = ps_mm.tile([P, 512], F32, tag="mm", name="pmm")[:, : ce - cs]
                        for k in range(KD):
                            wt = wq_tiles[k // wq_half]
                            nc.tensor.matmul(pt[:], xq1s[t][:, k, :],
                                             wt[:, k % wq_half, cs:ce],
                                             start=(k == 0), stop=(k == KD - 1))
                        segs = []
                        if cs < DIM:
                            segs.append((cs, min(ce, DIM), "q"))
                        if ce > DIM and cs < 2 * DIM:
                            segs.append((max(cs, DIM), min(ce, 2 * DIM), "k"))
                        if ce > 2 * DIM:
                            segs.append((max(cs, 2 * DIM), ce, "v"))
                        for (s0, s1, kind) in segs:
                            po = pt[:, s0 - cs:s1 - cs]
                            if kind == "q":
                                nc.scalar.activation(q_tm[:, s0:s1], po, AF.Identity,
                                                     scale=cq8s[t][:])
                            elif kind == "k":
                                nc.scalar.activation(k_tm[:, s0 - DIM:s1 - DIM], po,
                                                     AF.Identity, scale=c_list[t][:])
                            else:
                                h0 = (s0 - 2 * DIM) // HD
                                h1 = (s1 - 2 * DIM) // HD
                                nc.scalar.activation(
                                    v_aug[:, t, h0:h1, 0:HD], po, AF.Identity,
                                    scale=c_list[t][:])

                def p2_tp(t):
                    # 12 PE transposes batched 4-per-PSUM-bank, 3 DVE copies each dst
                    for half, src in ((0, q_tms[t]), (1, k_tms[t])):
                        fm = q_fm if half == 0 else k_fm
                        for g0 in range(0, KD, 4):
                            gn = min(4, KD - g0)
                            ptb = ps_tp.tile([P, 512], F32, tag="tp", name="ptb")[:, : gn * P]
                            for j in range(gn):
                                nc.tensor.transpose(ptb[:, j * P:(j + 1) * P],
                                                    src[:, (g0 + j) * P:(g0 + j + 1) * P],
                                                    idf[:])
                            nc.vector.tensor_copy(
                                fm[:, g0:g0 + gn, t * P:(t + 1) * P], ptb[:])

                with nc.named_scope(f"b{b}_qkv"):
                    for t in range(NT):
                        p2_mm(t)
                        if t >= 1:
                            p2_tp(t - 1)
                    p2_tp(NT - 1)

                # --- phase 3: attention, heads pipelined by one ---
                wp_tiles, wp_half = load_w(wproj_d, b, KD, DIM, 2)
                pairs = [(img, hh) for img in range(2) for hh in range(HEADS)]
                eTs = {}

                def attn_lt(img, hh):
                    po = (hh % 2) * HD
                    ch = hh // 2
                    lt = ps_lt.tile([P, 2, NTOK], F32, tag="lt")
                    for mt in range(2):
                        nc.tensor.matmul(
                            lt[:, mt, :],
                            k_fm[po:po + HD, ch, img * NTOK + mt * P: img * NTOK + (mt + 1) * P],
                            q_fm[po:po + HD, ch, img * NTOK: (img + 1) * NTOK],
                            start=True, stop=True)
                    eT = eTp.tile([P, 2, NTOK], F32, tag="eT")
                    nc.scalar.activation(eT[:], lt[:], AF.Exp)
                    eTs[(img, hh)] = eT

                def attn_oa(img, hh):
                    eT = eTs.pop((img, hh))
                    for nt in range(2):
                        oa = ps_oa.tile([P, HD + 1], F32, tag="oa")
                        for mt in range(2):
                            nc.tensor.matmul(
                                oa[:], eT[:, mt, nt * P:(nt + 1) * P],
                                v_aug[:, img * 2 + mt, hh, :],
                                start=(mt == 0), stop=(mt == 1))
                        rinv = scp.tile([P, 1], F32, tag="sc")
                        nc.vector.reciprocal(rinv[:], oa[:, HD:HD + 1])
                        nc.scalar.activation(
                            o_tm[:, img * 2 + nt, hh * HD:(hh + 1) * HD],
                            oa[:, 0:HD], AF.Identity, scale=rinv[:])

                with nc.named_scope(f"b{b}_attn"):
                    for i in range(len(pairs) + 1):
                        if i < len(pairs):
                            attn_lt(*pairs[i])
                        if i > 0:
                            attn_oa(*pairs[i - 1])

                # --- o-quant + proj + n2 chain (pipelined) ---
                xqo = [None] * NT
                cps = [None] * NT

                def o_quant(t):
                    s127, c = quant_smalls(o_tm[:, t, :], 4 * b + 1)
                    cps[t] = c
                    xqo[t] = quant_round_dma(o_tm[:, t, :], KD, s127)

                with nc.named_scope(f"b{b}_oq"):
                    for t in range(NT):
                        o_quant(t)

                wf1_tiles, wf1_half = load_w(wfc1_d, b, KD, HID, 3)
                xq2 = [None] * NT
                c3s = [None] * NT
                ssq2 = [None] * NT
                s1272 = [None] * NT
                h2s = [None] * NT

                def n2a(t):
                    sq = tmp_.tile([P, DIM], F32, tag="tm", name="sqn")
                    sv = scp.tile([P, 1], F32, tag="sc", name="ssqn")
                    nc.scalar.activation(sq[:], z[:, t, :], AF.Square, accum_out=sv[:])
                    ssq2[t] = sv

                def n2b(t):
                    rst = rstd_from_ssq(ssq2[t])
                    h = hp.tile([P, DIM], F32, tag="h")
                    h2s[t] = h
                    norm_mod(t, mt2, rst, h[:])
                    s127, c = quant_smalls(h[:], 4 * b + 2)
                    c3s[t] = c
                    s1272[t] = s127

                def n2c(t):
                    xq2[t] = quant_round_dma(h2s[t][:], KD, s1272[t])

                with nc.named_scope(f"b{b}_proj"):
                    for t in range(NT):
                        for (cs, ce) in _mm_chunks(DIM):
                            pt = ps_mm.tile([P, 512], F32, tag="mm", name="pmm")[:, : ce - cs]
                            for k in range(KD):
                                wt = wp_tiles[k // wp_half]
                                nc.tensor.matmul(pt[:], xqo[t][:, k, :],
                                                 wt[:, k % wp_half, cs:ce],
                                                 start=(k == 0), stop=(k == KD - 1))
                            # fused evac+residual: z += c * psum
                            nc.vector.scalar_tensor_tensor(
                                z[:, t, cs:ce], pt[:], cps[t][:], z[:, t, cs:ce],
                                OP.mult, OP.add)
                        n2a(t)
                        n2b(t)
                        n2c(t)

                # --- phase 5: fc1 + gelu + g-quant ---
                wf2_tiles, wf2_half = load_w(wfc2_d, b, KH, DIM, 3)
                xqg = [None] * NT
                c4s = [None] * NT
                gs = [None] * NT

                def gquant(t):
                    gh0, gh1 = gs[t]
                    am = scp.tile([P, 1], F32, tag="sc", name="am0")
                    nc.vector.tensor_reduce(am[:], gh0[:], axis=AX.X, op=OP.max,
                                            apply_absolute_value=True)
                    am1 = scp.tile([P, 1], F32, tag="sc", name="am1")
                    nc.vector.tensor_reduce(am1[:], gh1[:], axis=AX.X, op=OP.max,
                                            apply_absolute_value=True)
                    ac = scp.tile([P, 1], F32, tag="sc", name="amaxc")
                    nc.vector.tensor_tensor(ac[:], am[:], am1[:], OP.max)
                    ac2 = scp.tile([P, 1], F32, tag="sc", name="amaxc2")
                    nc.vector.tensor_scalar_max(ac2[:], ac[:], 1e-5)
                    rs = scp.tile([P, 1], F32, tag="sc", name="rcp")
                    nc.vector.reciprocal(rs[:], ac2[:])
                    s127 = scp.tile([P, 1], F32, tag="sc", name="s127")
                    nc.vector.tensor_scalar_mul(s127[:], rs[:], 127.0)
                    c = scp.tile([P, 1], F32, tag="sc", name="cc")
                    nc.vector.tensor_scalar(c[:], ac2[:], wsb[:, 4 * b + 3:4 * b + 4],
                                            None, OP.mult)
                    c4s[t] = c
                    xqg[t] = quant_round_dma_hid(gs[t], s127)

                with nc.named_scope(f"b{b}_fc1"):
                    for t in range(NT):
                        gh0 = gp.tile([P, HID // 2], F32, tag="g")
                        gh1 = gp.tile([P, HID // 2], F32, tag="g")
                        gs[t] = (gh0, gh1)
                        for ci, (cs, ce) in enumerate(_mm_chunks(HID)):
                            pt = ps_mm.tile([P, 512], F32, tag="mm", name="pmm")[:, : ce - cs]
                            for k in range(KD):
                                wt = wf1_tiles[k // wf1_half]
                                nc.tensor.matmul(pt[:], xq2[t][:, k, :],
                                                 wt[:, k % wf1_half, cs:ce],
                                                 start=(k == 0), stop=(k == KD - 1))
                            gh = gh0 if ci < 3 else gh1
                            off = cs - (0 if ci < 3 else HID // 2)
                            nc.scalar.activation(gh[:, off:off + ce - cs], pt[:],
                                                 AF.Gelu_apprx_tanh, scale=c3s[t][:])
                        if t > 0:
                            gquant(t - 1)
                    gquant(NT - 1)

                # --- phase 6: fc2 + residual, fused with next block's phase 1 ---
                fuse = b + 1 < depth
                if fuse:
                    mt1_nxt = load_mods(b + 1, 0, "mt1")
                    xq1_cur = [None] * NT
                    cq8_cur = [None] * NT
                    c_cur = [None] * NT
                    ssq_n = [None] * NT
                    h1s = [None] * NT
                    s127_n = [None] * NT

                def p1a(t):
                    sq = tmp_.tile([P, DIM], F32, tag="tm", name="sqn")
                    sv = scp.tile([P, 1], F32, tag="sc", name="ssqn")
                    nc.scalar.activation(sq[:], z[:, t, :], AF.Square, accum_out=sv[:])
                    ssq_n[t] = sv

                def p1b(t):
                    rst = rstd_from_ssq(ssq_n[t])
                    h = hp.tile([P, DIM], F32, tag="h")
                    h1s[t] = h
                    norm_mod(t, mt1_nxt, rst, h[:])
                    s127, c = quant_smalls(h[:], 4 * (b + 1))
                    c_cur[t] = c
                    cq8 = scp.tile([P, 1], F32, tag="sc", name="cq8")
                    nc.vector.tensor_scalar_mul(cq8[:], c[:], 0.125)
                    cq8_cur[t] = cq8
                    s127_n[t] = s127

                def p1c(t):
                    xq1_cur[t] = quant_round_dma(h1s[t][:], KD, s127_n[t])

                with nc.named_scope(f"b{b}_fc2"):
                    for t in range(NT):
                        for (cs, ce) in _mm_chunks(DIM):
                            pt = ps_mm.tile([P, 512], F32, tag="mm", name="pmm")[:, : ce - cs]
                            for k in range(KH):
                                wt = wf2_tiles[k // wf2_half]
                                nc.tensor.matmul(pt[:], xqg[t][:, k, :],
                                                 wt[:, k % wf2_half, cs:ce],
                                                 start=(k == 0), stop=(k == KH - 1))
                            nc.vector.scalar_tensor_tensor(
                                z[:, t, cs:ce], pt[:], c4s[t][:], z[:, t, cs:ce],
                                OP.mult, OP.add)
                        if fuse:
                            p1a(t)
                            p1b(t)
                            p1c(t)

            # ---------------- final norm + head ----------------
            with nc.named_scope("head"):
                hw_pieces = []
                for i in range(3):
                    hwp = gp.tile([P, 2, DIM], F32, tag="g", name="hwp")
                    nc.gpsimd.dma_start(
                        hwp[:], headWT_d[i * 2 * P:(i + 1) * 2 * P, :].rearrange(
                            "(o p) d -> p o d", p=P))
                    hw_pieces.append(hwp)
                hbrow = tmp_.tile([1, DIM], F32, tag="tm", name="hbrow")
                nc.sync.dma_start(hbrow[:], headb_d[:])
                hbb = gp.tile([P, DIM], F32, tag="g", name="hbb")
                nc.gpsimd.partition_broadcast(hbb[:], hbrow[0:1, :])
                ssq_f = [None] * NT
                sqf = tmp_.tile([P, DIM], F32, tag="tm", name="sqf")
                for t in range(NT):
                    sv = scp.tile([P, 1], F32, tag="sc", name="ssqf")
                    nc.scalar.activation(sqf[:], z[:, t, :], AF.Square, accum_out=sv[:])
                    ssq_f[t] = sv
                for t in range(NT):
                    rst = rstd_from_ssq(ssq_f[t])
                    zn = hp.tile([P, DIM], F32, tag="h")
                    nc.vector.tensor_scalar_mul(zn[:], z[:, t, :], rst[:])
                    znT = hp.tile([P, DIM], F32, tag="h")
                    for g0 in range(0, KD, 4):
                        gn = min(4, KD - g0)
                        ptb = ps_tp.tile([P, 512], F32, tag="tp", name="ptb")[:, : gn * P]
                        for j in range(gn):
                            nc.tensor.transpose(ptb[:, j * P:(j + 1) * P],
                                                zn[:, (g0 + j) * P:(g0 + j + 1) * P], idf[:])
                        nc.vector.tensor_copy(znT[:, g0 * P:(g0 + gn) * P], ptb[:])
                    for (cs, ce) in _mm_chunks(DIM):
                        pt = ps_mm.tile([P, 512], F32, tag="mm", name="pmm")[:, : ce - cs]
                        for k in range(KD):
                            nc.tensor.matmul(pt[:], znT[:, k * P:(k + 1) * P],
                                             hw_pieces[k // 2][:, k % 2, cs:ce],
                                             start=(k == 0), stop=(k == KD - 1))
                        ot = tmp_.tile([P, DIM], F32, tag="tm", name="ot")[:, : ce - cs]
                        nc.vector.tensor_tensor(ot[:], pt[:], hbb[:, cs:ce], OP.add)
                        nc.sync.dma_start(out_d[t * P:(t + 1) * P, cs:ce], ot[:])

    nc.compile()
    return nc


# ---------------------------------------------------------------------------
# host-side numerics (numpy, fp32 — matches jax CPU within ~1e-7)

def _gelu_tanh(x):
    x = x.astype(np.float32)
    c = np.float32(math.sqrt(2.0 / math.pi))
    return np.float32(0.5) * x * (np.float32(1.0) +
                                  np.tanh(c * (x + np.float32(0.044715) * x * x * x)))


def _time_embedding(t, t_w1, t_b1, t_w2, t_b2):
    half = DIM // 2
    freqs = np.exp(-np.log(10000.0) * np.arange(half, dtype=np.float32) / (half - 1)).astype(np.float32)
    args = t[:, None].astype(np.float32) * freqs[None, :]
    emb = np.concatenate([np.sin(args), np.cos(args)], axis=-1).astype(np.float32)
    h = _gelu_tanh(emb @ t_w1.T + t_b1)
    return (h @ t_w2.T + t_b2).astype(np.float32)


def _quant_w(w):
    ws = np.float32(np.mean(np.abs(w), dtype=np.float64)) + np.float32(1e-5)
    wq = np.clip(np.round(w.astype(np.float32) / ws), -1.0, 1.0)
    return wq, ws


def _prepare(inputs):
    x = np.asarray(inputs["x"], np.float32)
    t = np.asarray(inputs["t"], np.float32)
    B = x.shape[0]
    n_cores = 8
    per = B // n_cores  # 2
    p = PATCH
    hh = IMG // p

    xp = x.reshape(B, CIN, hh, p, hh, p).transpose(0, 2, 4, 1, 3, 5).reshape(B, hh * hh, CIN * p * p)

    t_emb = _time_embedding(t, inputs["t_w1"], inputs["t_b1"], inputs["t_w2"], inputs["t_b2"])
    silu = (t_emb / (1.0 + np.exp(-t_emb))).astype(np.float32)

    depth = DEPTH
    mods = np.zeros((depth, 2, B, 2, DIM), np.float32)  # [blk, norm, img, A/B, D]
    wscl = np.zeros((4 * depth,), np.float32)
    wq_all, wp_all, wf1_all, wf2_all = [], [], [], []
    for b in range(depth):
        mod = silu @ np.asarray(inputs["blk_ada_w"][b], np.float32).T + np.asarray(
            inputs["blk_ada_b"][b], np.float32)
        sh1, sc1, sh2, sc2 = np.split(mod, 4, axis=-1)
        n1 = np.asarray(inputs["blk_norm1"][b], np.float32)
        n2 = np.asarray(inputs["blk_norm2"][b], np.float32)
        mods[b, 0, :, 0, :] = n1[None, :] * (1.0 + sc1)
        mods[b, 0, :, 1, :] = sh1
        mods[b, 1, :, 0, :] = n2[None, :] * (1.0 + sc2)
        mods[b, 1, :, 1, :] = sh2

        for j, (nm, lst) in enumerate([("blk_qkv", wq_all), ("blk_proj", wp_all),
                                       ("blk_fc1", wf1_all), ("blk_fc2", wf2_all)]):
            wq, ws = _quant_w(np.asarray(inputs[nm][b], np.float32))
            lst.append(np.ascontiguousarray(wq.T).astype(ml_dtypes.float8_e4m3))
            wscl[4 * b + j] = ws / np.float32(127.0)

    wqkv = np.stack(wq_all)
    wproj = np.stack(wp_all)
    wfc1 = np.stack(wf1_all)
    wfc2 = np.stack(wf2_all)

    posb = (np.asarray(inputs["pos_embed"][0], np.float32) +
            np.asarray(inputs["patch_b"], np.float32)[None, :]).astype(np.float32)
    patchWT = np.ascontiguousarray(np.asarray(inputs["patch_w"], np.float32).T)
    norm_w = np.asarray(inputs["norm_w"], np.float32)
    headWT = np.ascontiguousarray(np.asarray(inputs["head_w"], np.float32).T * norm_w[:, None])
    headb = np.asarray(inputs["head_b"], np.float32)[None, :]

    key = ("prog", depth)
    if key not in _CACHED:
        _CACHED[key] = build_program(depth)
    nc = _CACHED[key]

    in_maps = []
    for c in range(n_cores):
        imgs = slice(c * per, (c + 1) * per)
        xpT = np.ascontiguousarray(xp[imgs].reshape(per * hh * hh, CIN * p * p).T)
        in_maps.append(dict(
            xpT=xpT, posb=posb, patchWT=patchWT, headWT=headWT, headb=headb,
            wqkv=wqkv, wproj=wproj, wfc1=wfc1, wfc2=wfc2,
            mods=np.ascontiguousarray(
                np.broadcast_to(mods[:, :, None, imgs], (depth, 2, 128, per, 2, DIM))),
            wscl=wscl[None, :],
        ))

    return nc, in_maps


def _assemble(res, B=16, per=2):
    p = PATCH
    hh = IMG // p
    out = np.zeros((B, CIN, IMG, IMG), np.float32)
    for c in range(B // per):
        zo = res.results[c]["zout"]  # [512, 768]
        for i in range(per):
            zi = zo[i * 256:(i + 1) * 256]
            out[c * per + i] = zi.reshape(hh, hh, CIN, p, p).transpose(2, 0, 3, 1, 4).reshape(CIN, IMG, IMG)
    return out


def kernel(**inputs):
    nc, in_maps = _prepare(inputs)
    res = run_bass_kernel_spmd(nc, in_maps, list(range(len(in_maps))), trace=False)
    return _assemble(res)
